# revision 13
# baseline (speedup 1.0000x reference)
"""Trainium2 Bass kernel for nn_BlockGatingUnit.

Reference computation (per batch element b of x [8, 256, 256, 256] f32):
    u, v = split(x, 2, axis=1)                  # each [128, 256, 256]
    v    = LayerNorm(v) over all non-batch dims (affine = identity)
    y    = v @ W.T + b                          # Linear along last dim
    out  = u * (y + 1)                          # [8, 128, 256, 256]

Sharding: pure data-parallel — batch dim 8 across the 8 NeuronCores, one
batch element per core, W/b replicated.  LayerNorm stats are per batch
element, so no collectives are needed.

Per-core plan (memory-bound; HBM floor = read 67MB + write 33.5MB ~ 281us
at ~358 GB/s).  LayerNorm is an affine map, so it commutes with the
Linear layer:

    out = u * (LN(v) @ W.T + b + 1)
        = (u * inv_std) * (v @ W.T + beta'),
    beta'[o] = (b[o] + 1) * std - mean * sum_w W[o, w]

which lets the matmul run on RAW (unnormalized) bf16 v and pushes all of
LayerNorm into one per-column scalar and one bf16 bias row.

  Phase 1:  casting gpsimd DMA streams v (f32 HBM -> bf16 SBUF tiles);
            bn_stats on VectorE accumulates LN stats; TensorE transposes
            each 128x128 block (w onto partitions, PE otherwise idle in
            this phase); one grouped [128,1024] PSUM->SBUF copy per tile
            on ScalarE lands the transposed bf16 v in a persistent 16.8MB
            SBUF buffer.  v is read from HBM exactly once.
  Stats:    bn_aggr + cross-partition reduce via tiny ones-matmuls ->
            inv_std column + beta' row.
  Phase 2:  pure-matmul stream on TensorE (stays at the fast p-state):
            per 128-row group, 2 bf16 matmuls against W.T chunks + a K=1
            ones-row matmul adding beta', accumulated in PSUM f32; the
            epilogue is a single fused VectorE op
            out = (u * inv_std) * y_psum; streamed u in / out via DMA.
"""

import sys

for _p in ("/opt/trn_rl_repo", "/root/.axon_site/_ro/trn_rl_repo"):
    if _p not in sys.path:
        sys.path.append(_p)

import numpy as np

import concourse.bass as bass
import concourse.tile as tile
from concourse import mybir
from concourse.masks import make_identity

F32 = mybir.dt.float32
BF16 = mybir.dt.bfloat16

EPS = 1e-5

# Per-core shard shapes (hardcoded; batch dim 8 == n_cores).
C2, G, Wd = 256, 256, 256          # x shard [C2, G, Wd]
C = C2 // 2                        # u/v channel count
ROWS = C * G                       # 32768 rows of length Wd
P = 128                            # partitions
FPT = 4                            # rows per partition per tile
TILE_ROWS = P * FPT                # 512 rows -> 1MB f32 tiles
NT = ROWS // TILE_ROWS             # 64 tiles
NCORES = 8


def build_bass():
    nc = bass.Bass()

    x_h = nc.declare_dram_parameter("x", [C2, G, Wd], F32, isOutput=False)
    w_h = nc.declare_dram_parameter("W", [Wd, Wd], F32, isOutput=False)
    b_h = nc.declare_dram_parameter("b", [Wd], F32, isOutput=False)
    o_h = nc.declare_dram_parameter("out", [C, G, Wd], F32, isOutput=True)

    x_ap = x_h[:, :, :]
    # [t, p, f, w] tiling: row = t*512 + p*4 + f, contiguous 1MB per tile.
    u_t = x_ap[0:C].rearrange("c g w -> (c g) w").rearrange(
        "(t p f) w -> t p f w", p=P, f=FPT
    )
    v_t = x_ap[C:C2].rearrange("c g w -> (c g) w").rearrange(
        "(t p f) w -> t p f w", p=P, f=FPT
    )
    out_t = o_h[:, :, :].rearrange("c g w -> (c g) w").rearrange(
        "(t p f) w -> t p f w", p=P, f=FPT
    )

    with tile.TileContext(nc) as tc:
        with (
            tc.tile_pool(name="persist", bufs=1) as persist,
            tc.tile_pool(name="consts", bufs=1) as consts,
            tc.tile_pool(name="vload", bufs=4) as vload,
            tc.tile_pool(name="uload", bufs=6) as uload,
            tc.tile_pool(name="ostore", bufs=6) as ostore,
            tc.tile_pool(name="ps", bufs=4, space="PSUM") as psall,
        ):
            # ---- constants -------------------------------------------------
            ident = consts.tile([P, P], BF16)
            make_identity(nc, ident)

            ones_col_f = consts.tile([P, 1], F32)
            nc.vector.memset(ones_col_f, 1.0)
            ones_row_f = consts.tile([1, P], F32)
            nc.vector.memset(ones_row_f, 1.0)
            ones_col_b = consts.tile([P, 1], BF16)
            nc.vector.memset(ones_col_b, 1.0)
            eps_col = consts.tile([P, 1], F32)
            nc.vector.memset(eps_col, EPS)

            # W.T in bf16: wt_bf[:, k, o] = W[o, k*128 + w_local].
            w_f32 = consts.tile([P, 2, Wd], F32)
            nc.sync.dma_start(
                out=w_f32, in_=w_h[:, :].rearrange("(m p) w -> p m w", p=P)
            )
            w_bf = consts.tile([P, 2, Wd], BF16)
            nc.scalar.copy(w_bf, w_f32)
            wt_bf = consts.tile([P, 2, Wd], BF16)
            for m in range(2):
                for k in range(2):
                    ps_w = psall.tile([P, P], F32, tag="ps")
                    # transpose as a REGULAR matmul (w_chunk.T @ I): counts
                    # as PE-busy for the clock boost, FWL-eligible LDW.
                    nc.tensor.matmul(
                        ps_w,
                        lhsT=w_bf[:, m, k * P : (k + 1) * P],
                        rhs=ident,
                        start=True,
                        stop=True,
                    )
                    nc.scalar.copy(wt_bf[:, k, m * P : (m + 1) * P], ps_w)

            # Row sums of W (= column sums of W.T): ones @ WT.
            ps_sw = psall.tile([1, Wd], F32, tag="ps")
            nc.tensor.matmul(
                ps_sw, lhsT=ones_col_b, rhs=wt_bf[:, 0, :], start=True, stop=False
            )
            nc.tensor.matmul(
                ps_sw, lhsT=ones_col_b, rhs=wt_bf[:, 1, :], start=False, stop=True
            )
            sumw_row = consts.tile([1, Wd], F32)
            nc.vector.tensor_copy(sumw_row, ps_sw)

            # b + 1 (f32 row).
            b_f32 = consts.tile([1, Wd], F32)
            nc.sync.dma_start(out=b_f32, in_=b_h[None, :])
            bp1_row = consts.tile([1, Wd], F32)
            nc.scalar.activation(
                bp1_row, b_f32, mybir.ActivationFunctionType.Identity, bias=1.0
            )

            # ---- persistent buffers ---------------------------------------
            # Transposed bf16 v: [w_local, t, f, k, r] with w on partitions.
            vT = persist.tile([P, NT, FPT, 2, P], BF16)        # 16.8 MB
            stats = persist.tile([P, NT, 2, 6], F32)           # bn_stats out

            # ---- phase 1: cast-load, stats, transpose ---------------------
            for t in range(NT):
                v_in = vload.tile([P, FPT, Wd], BF16, tag="v")
                # gpsimd (SWDGE) DMA casts f32 -> bf16 in the datapath.
                nc.gpsimd.dma_start(out=v_in, in_=v_t[t])
                # bn_stats free-size limit is 512 -> two calls of 512 each.
                nc.vector.bn_stats(
                    out=stats[:, t, 0, :],
                    in_=v_in[:, 0:2, :].rearrange("p f w -> p (f w)"),
                )
                nc.vector.bn_stats(
                    out=stats[:, t, 1, :],
                    in_=v_in[:, 2:4, :].rearrange("p f w -> p (f w)"),
                )
                vt_ps = psall.tile([P, FPT, 2, P], F32, tag="ps")
                for f in range(FPT):
                    for k in range(2):
                        nc.tensor.matmul(
                            vt_ps[:, f, k, :],
                            lhsT=v_in[:, f, k * P : (k + 1) * P],
                            rhs=ident,
                            start=True,
                            stop=True,
                        )
                nc.scalar.copy(vT[:, t], vt_ps)

            # ---- stats finalize -------------------------------------------
            mvm = consts.tile([P, 3], F32)
            nc.vector.bn_aggr(
                out=mvm[:, 0:2],
                in_=stats[:, :, :, :].rearrange("p t f s -> p (t f) s"),
            )
            nc.vector.tensor_mul(mvm[:, 2:3], mvm[:, 0:1], mvm[:, 0:1])

            # Cross-partition: totals = ones_col.T @ [mean, var, mean^2],
            # then broadcast back to all partitions via ones_row.T @ totals.
            ps_tot = psall.tile([1, 3], F32, tag="ps")
            nc.tensor.matmul(
                ps_tot, lhsT=ones_col_f, rhs=mvm, start=True, stop=True
            )
            row_tot = consts.tile([1, 3], F32)
            nc.vector.tensor_copy(row_tot, ps_tot)
            ps_bc = psall.tile([P, 3], F32, tag="ps")
            nc.tensor.matmul(
                ps_bc, lhsT=ones_row_f, rhs=row_tot, start=True, stop=True
            )
            tot = consts.tile([P, 3], F32)
            nc.vector.tensor_copy(tot, ps_bc)

            mean_c = consts.tile([P, 1], F32)
            nc.vector.tensor_scalar_mul(mean_c, tot[:, 0:1], 1.0 / P)
            ex2_c = consts.tile([P, 1], F32)
            nc.vector.tensor_add(ex2_c, tot[:, 1:2], tot[:, 2:3])
            nc.vector.tensor_scalar_mul(ex2_c, ex2_c, 1.0 / P)
            msq_c = consts.tile([P, 1], F32)
            nc.vector.tensor_mul(msq_c, mean_c, mean_c)
            var_c = consts.tile([P, 1], F32)
            nc.vector.tensor_sub(var_c, ex2_c, msq_c)
            std_c = consts.tile([P, 1], F32)
            nc.scalar.activation(
                std_c, var_c, mybir.ActivationFunctionType.Sqrt, bias=eps_col
            )
            inv_std_c = consts.tile([P, 1], F32)
            nc.vector.reciprocal(inv_std_c, std_c)

            # beta'[o] = (b[o] + 1) * std - mean * sumW[o]   (bf16 row)
            beta_f = consts.tile([1, Wd], F32)
            nc.vector.tensor_scalar_mul(beta_f, bp1_row, std_c[0:1, :])
            tmp_row = consts.tile([1, Wd], F32)
            nc.vector.tensor_scalar_mul(tmp_row, sumw_row, mean_c[0:1, :])
            nc.vector.tensor_sub(beta_f, beta_f, tmp_row)
            ps_bb = psall.tile([P, Wd], F32, tag="ps")
            nc.tensor.matmul(
                ps_bb, lhsT=ones_row_f, rhs=beta_f, start=True, stop=True
            )
            beta_bc = consts.tile([P, Wd], F32)
            nc.vector.tensor_copy(beta_bc, ps_bb)
            beta_ap = bass.AP(
                tensor=beta_bc[:, :].tensor,
                offset=beta_bc[:, :].offset,
                ap=[beta_bc[:, :].ap[0], [0, FPT], [1, Wd]],
            )

            # ---- phase 2: matmul + fused epilogue -------------------------
            for t in range(NT):
                u_in = uload.tile([P, FPT, Wd], F32, tag="u")
                nc.gpsimd.dma_start(out=u_in, in_=u_t[t])

                y_ps = psall.tile([P, FPT, Wd], F32, tag="ps")
                for f in range(FPT):
                    nc.tensor.matmul(
                        y_ps[:, f, :],
                        lhsT=vT[:, t, f, 0, :],
                        rhs=wt_bf[:, 0, :],
                        start=True,
                        stop=False,
                    )
                    nc.tensor.matmul(
                        y_ps[:, f, :],
                        lhsT=vT[:, t, f, 1, :],
                        rhs=wt_bf[:, 1, :],
                        start=False,
                        stop=True,
                    )
                # y += beta' (broadcast row), in place on PSUM.
                nc.vector.tensor_add(y_ps, y_ps, beta_ap)

                o_sb = ostore.tile([P, FPT, Wd], F32, tag="o")
                # out = (u * inv_std) * (z + beta')
                nc.vector.scalar_tensor_tensor(
                    out=o_sb,
                    in0=u_in,
                    scalar=inv_std_c,
                    in1=y_ps,
                    op0=mybir.AluOpType.mult,
                    op1=mybir.AluOpType.mult,
                )
                nc.gpsimd.dma_start(out=out_t[t], in_=o_sb)

    return nc


def split_multiwaits(nc):
    """Walrus in this toolchain accepts at most ONE sync-wait command per
    instruction.  Tile's semaphore assignment can emit several (e.g. a DMA
    slot-reuse waits on both the previous reader's engine sem and the old
    DMA's completion lane).  Hoist all but one wait into standalone
    InstEventSemaphore instructions on the same engine stream immediately
    before the instruction — semantically identical (the sequencer performs
    the waits in order before dispatching)."""
    n_split = 0
    for f in nc.m.functions:
        for blk in f.blocks:
            new_insts = []
            for inst in blk.instructions:
                si = getattr(inst, "sync_info", None)
                if si is not None and si.on_wait and len(si.on_wait) > 1:
                    waits = list(si.on_wait)
                    for j, w in enumerate(waits[:-1]):
                        wi = mybir.InstEventSemaphore(
                            name=f"{inst.name}-hw{j}",
                            engine=inst.engine,
                            ins=[],
                            outs=[],
                        )
                        wi.sync_info = mybir.SyncInfo(on_wait=[w], on_update=[])
                        new_insts.append(wi)
                        n_split += 1
                    inst.sync_info = mybir.SyncInfo(
                        on_wait=[waits[-1]], on_update=list(si.on_update or [])
                    )
                new_insts.append(inst)
            blk.instructions[:] = new_insts
    return n_split


_NC_CACHE = None


def _get_nc():
    global _NC_CACHE
    if _NC_CACHE is None:
        nc = build_bass()
        split_multiwaits(nc)
        _NC_CACHE = nc
    return _NC_CACHE


def run(inputs, trace=False, **spmd_kwargs):
    from concourse.bass_utils import run_bass_kernel_spmd

    x = np.ascontiguousarray(np.asarray(inputs["x"], dtype=np.float32))
    W = np.ascontiguousarray(np.asarray(inputs["W"], dtype=np.float32))
    b = np.ascontiguousarray(np.asarray(inputs["b"], dtype=np.float32))
    assert x.shape == (NCORES, C2, G, Wd), x.shape

    nc = _get_nc()
    in_maps = [{"x": x[i], "W": W, "b": b} for i in range(NCORES)]
    res = run_bass_kernel_spmd(
        nc, in_maps, core_ids=list(range(NCORES)), trace=trace, **spmd_kwargs
    )
    out = np.stack([res.results[i]["out"] for i in range(NCORES)], axis=0)
    return out, res


def kernel(**inputs) -> np.ndarray:
    out, _ = run(inputs)
    return out


# revision 14
# speedup vs baseline: 1.2205x; 1.2205x over previous
"""Trainium2 Bass kernel for nn_BlockGatingUnit.

Reference computation (per batch element b of x [8, 256, 256, 256] f32):
    u, v = split(x, 2, axis=1)                  # each [128, 256, 256]
    v    = LayerNorm(v) over all non-batch dims (affine = identity)
    y    = v @ W.T + b                          # Linear along last dim
    out  = u * (y + 1)                          # [8, 128, 256, 256]

Sharding: pure data-parallel — batch dim 8 across the 8 NeuronCores, one
batch element per core, W/b replicated.  LayerNorm stats are per batch
element, so no collectives are needed.

Per-core plan (memory-bound; HBM floor = read 67MB + write 33.5MB ~ 281us
at ~358 GB/s).  LayerNorm is an affine map, so it commutes with the
Linear layer:

    out = u * (LN(v) @ W.T + b + 1)
        = (u * inv_std) * (v @ W.T + beta'),
    beta'[o] = (b[o] + 1) * std - mean * sum_w W[o, w]

which lets the matmul run on RAW (unnormalized) bf16 v and pushes all of
LayerNorm into one per-column scalar and one bf16 bias row.

  Phase 1:  casting gpsimd DMA streams v (f32 HBM -> bf16 SBUF tiles);
            bn_stats on VectorE accumulates LN stats; TensorE transposes
            each 128x128 block (w onto partitions, PE otherwise idle in
            this phase); one grouped [128,1024] PSUM->SBUF copy per tile
            on ScalarE lands the transposed bf16 v in a persistent 16.8MB
            SBUF buffer.  v is read from HBM exactly once.
  Stats:    bn_aggr + cross-partition reduce via tiny ones-matmuls ->
            inv_std column + beta' row.
  Phase 2:  pure-matmul stream on TensorE (stays at the fast p-state):
            per 128-row group, 2 bf16 matmuls against W.T chunks + a K=1
            ones-row matmul adding beta', accumulated in PSUM f32; the
            epilogue is a single fused VectorE op
            out = (u * inv_std) * y_psum; streamed u in / out via DMA.
"""

import sys

for _p in ("/opt/trn_rl_repo", "/root/.axon_site/_ro/trn_rl_repo"):
    if _p not in sys.path:
        sys.path.append(_p)

import numpy as np

import concourse.bass as bass
import concourse.tile as tile
from concourse import mybir
from concourse.masks import make_identity

F32 = mybir.dt.float32
BF16 = mybir.dt.bfloat16

EPS = 1e-5

# Per-core shard shapes (hardcoded; batch dim 8 == n_cores).
C2, G, Wd = 256, 256, 256          # x shard [C2, G, Wd]
C = C2 // 2                        # u/v channel count
ROWS = C * G                       # 32768 rows of length Wd
P = 128                            # partitions
FPT = 4                            # rows per partition per tile
TILE_ROWS = P * FPT                # 512 rows -> 1MB f32 tiles
NT = ROWS // TILE_ROWS             # 64 tiles
NCORES = 8


def build_bass():
    nc = bass.Bass()

    x_h = nc.declare_dram_parameter("x", [C2, G, Wd], F32, isOutput=False)
    w_h = nc.declare_dram_parameter("W", [Wd, Wd], F32, isOutput=False)
    b_h = nc.declare_dram_parameter("b", [Wd], F32, isOutput=False)
    o_h = nc.declare_dram_parameter("out", [C, G, Wd], F32, isOutput=True)

    x_ap = x_h[:, :, :]
    # [t, p, f, w] tiling: row = t*512 + p*4 + f, contiguous 1MB per tile.
    u_t = x_ap[0:C].rearrange("c g w -> (c g) w").rearrange(
        "(t p f) w -> t p f w", p=P, f=FPT
    )
    v_t = x_ap[C:C2].rearrange("c g w -> (c g) w").rearrange(
        "(t p f) w -> t p f w", p=P, f=FPT
    )
    out_t = o_h[:, :, :].rearrange("c g w -> (c g) w").rearrange(
        "(t p f) w -> t p f w", p=P, f=FPT
    )

    with tile.TileContext(nc) as tc:
        with (
            tc.tile_pool(name="persist", bufs=1) as persist,
            tc.tile_pool(name="consts", bufs=1) as consts,
            tc.tile_pool(name="vload", bufs=3) as vload,
            tc.tile_pool(name="vbf", bufs=3) as vbf,
            tc.tile_pool(name="uload", bufs=5) as uload,
            tc.tile_pool(name="ostore", bufs=5) as ostore,
            tc.tile_pool(name="ps", bufs=4, space="PSUM") as psall,
        ):
            # ---- constants -------------------------------------------------
            ident = consts.tile([P, P], BF16)
            make_identity(nc, ident)

            ones_col_f = consts.tile([P, 1], F32)
            nc.vector.memset(ones_col_f, 1.0)
            ones_row_f = consts.tile([1, P], F32)
            nc.vector.memset(ones_row_f, 1.0)
            ones_col_b = consts.tile([P, 1], BF16)
            nc.vector.memset(ones_col_b, 1.0)
            eps_col = consts.tile([P, 1], F32)
            nc.vector.memset(eps_col, EPS)

            # W.T in bf16: wt_bf[:, k, o] = W[o, k*128 + w_local].
            w_f32 = consts.tile([P, 2, Wd], F32)
            nc.sync.dma_start(
                out=w_f32, in_=w_h[:, :].rearrange("(m p) w -> p m w", p=P)
            )
            w_bf = consts.tile([P, 2, Wd], BF16)
            nc.scalar.copy(w_bf, w_f32)
            wt_bf = consts.tile([P, 2, Wd], BF16)
            for m in range(2):
                for k in range(2):
                    ps_w = psall.tile([P, P], F32, tag="ps")
                    # transpose as a REGULAR matmul (w_chunk.T @ I): counts
                    # as PE-busy for the clock boost, FWL-eligible LDW.
                    nc.tensor.matmul(
                        ps_w,
                        lhsT=w_bf[:, m, k * P : (k + 1) * P],
                        rhs=ident,
                        start=True,
                        stop=True,
                    )
                    nc.scalar.copy(wt_bf[:, k, m * P : (m + 1) * P], ps_w)

            # Row sums of W (= column sums of W.T): ones @ WT.
            ps_sw = psall.tile([1, Wd], F32, tag="ps")
            nc.tensor.matmul(
                ps_sw, lhsT=ones_col_b, rhs=wt_bf[:, 0, :], start=True, stop=False
            )
            nc.tensor.matmul(
                ps_sw, lhsT=ones_col_b, rhs=wt_bf[:, 1, :], start=False, stop=True
            )
            sumw_row = consts.tile([1, Wd], F32)
            nc.vector.tensor_copy(sumw_row, ps_sw)

            # b + 1 (f32 row).
            b_f32 = consts.tile([1, Wd], F32)
            nc.sync.dma_start(out=b_f32, in_=b_h[None, :])
            bp1_row = consts.tile([1, Wd], F32)
            nc.scalar.activation(
                bp1_row, b_f32, mybir.ActivationFunctionType.Identity, bias=1.0
            )

            # ---- persistent buffers ---------------------------------------
            # Transposed bf16 v: [w_local, t, f, k, r] with w on partitions.
            vT = persist.tile([P, NT, FPT, 2, P], BF16)        # 16.8 MB
            stats = persist.tile([P, NT, 2, 6], F32)           # bn_stats out

            # ---- phase 1: cast-load, stats, transpose ---------------------
            for t in range(NT):
                v_f = vload.tile([P, FPT, Wd], F32, tag="v")
                # f32 load on the fast HWDGE path (the casting SWDGE DMA
                # only sustains ~250 GB/s); convert on ACT/DVE (alternating)
                # which both have slack in this phase.
                nc.sync.dma_start(out=v_f, in_=v_t[t])
                v_in = vbf.tile([P, FPT, Wd], BF16, tag="vb")
                if t % 2 == 0:
                    nc.scalar.copy(v_in, v_f)
                else:
                    nc.vector.tensor_copy(v_in, v_f)
                # bn_stats free-size limit is 512 -> two calls of 512 each.
                nc.vector.bn_stats(
                    out=stats[:, t, 0, :],
                    in_=v_in[:, 0:2, :].rearrange("p f w -> p (f w)"),
                )
                nc.vector.bn_stats(
                    out=stats[:, t, 1, :],
                    in_=v_in[:, 2:4, :].rearrange("p f w -> p (f w)"),
                )
                vt_ps = psall.tile([P, FPT, 2, P], F32, tag="ps")
                for f in range(FPT):
                    for k in range(2):
                        nc.tensor.matmul(
                            vt_ps[:, f, k, :],
                            lhsT=v_in[:, f, k * P : (k + 1) * P],
                            rhs=ident,
                            start=True,
                            stop=True,
                        )
                nc.scalar.copy(vT[:, t], vt_ps)

            # ---- stats finalize -------------------------------------------
            mvm = consts.tile([P, 3], F32)
            nc.vector.bn_aggr(
                out=mvm[:, 0:2],
                in_=stats[:, :, :, :].rearrange("p t f s -> p (t f) s"),
            )
            nc.vector.tensor_mul(mvm[:, 2:3], mvm[:, 0:1], mvm[:, 0:1])

            # Cross-partition: totals = ones_col.T @ [mean, var, mean^2],
            # then broadcast back to all partitions via ones_row.T @ totals.
            ps_tot = psall.tile([1, 3], F32, tag="ps")
            nc.tensor.matmul(
                ps_tot, lhsT=ones_col_f, rhs=mvm, start=True, stop=True
            )
            row_tot = consts.tile([1, 3], F32)
            nc.vector.tensor_copy(row_tot, ps_tot)
            ps_bc = psall.tile([P, 3], F32, tag="ps")
            nc.tensor.matmul(
                ps_bc, lhsT=ones_row_f, rhs=row_tot, start=True, stop=True
            )
            tot = consts.tile([P, 3], F32)
            nc.vector.tensor_copy(tot, ps_bc)

            mean_c = consts.tile([P, 1], F32)
            nc.vector.tensor_scalar_mul(mean_c, tot[:, 0:1], 1.0 / P)
            ex2_c = consts.tile([P, 1], F32)
            nc.vector.tensor_add(ex2_c, tot[:, 1:2], tot[:, 2:3])
            nc.vector.tensor_scalar_mul(ex2_c, ex2_c, 1.0 / P)
            msq_c = consts.tile([P, 1], F32)
            nc.vector.tensor_mul(msq_c, mean_c, mean_c)
            var_c = consts.tile([P, 1], F32)
            nc.vector.tensor_sub(var_c, ex2_c, msq_c)
            std_c = consts.tile([P, 1], F32)
            nc.scalar.activation(
                std_c, var_c, mybir.ActivationFunctionType.Sqrt, bias=eps_col
            )
            inv_std_c = consts.tile([P, 1], F32)
            nc.vector.reciprocal(inv_std_c, std_c)

            # beta'[o] = (b[o] + 1) * std - mean * sumW[o]   (bf16 row)
            beta_f = consts.tile([1, Wd], F32)
            nc.vector.tensor_scalar_mul(beta_f, bp1_row, std_c[0:1, :])
            tmp_row = consts.tile([1, Wd], F32)
            nc.vector.tensor_scalar_mul(tmp_row, sumw_row, mean_c[0:1, :])
            nc.vector.tensor_sub(beta_f, beta_f, tmp_row)
            ps_bb = psall.tile([P, Wd], F32, tag="ps")
            nc.tensor.matmul(
                ps_bb, lhsT=ones_row_f, rhs=beta_f, start=True, stop=True
            )
            beta_bc = consts.tile([P, Wd], F32)
            nc.vector.tensor_copy(beta_bc, ps_bb)
            beta_ap = bass.AP(
                tensor=beta_bc[:, :].tensor,
                offset=beta_bc[:, :].offset,
                ap=[beta_bc[:, :].ap[0], [0, FPT], [1, Wd]],
            )

            # ---- phase 2: matmul + fused epilogue -------------------------
            for t in range(NT):
                u_in = uload.tile([P, FPT, Wd], F32, tag="u")
                nc.sync.dma_start(out=u_in, in_=u_t[t])

                y_ps = psall.tile([P, FPT, Wd], F32, tag="ps")
                for f in range(FPT):
                    nc.tensor.matmul(
                        y_ps[:, f, :],
                        lhsT=vT[:, t, f, 0, :],
                        rhs=wt_bf[:, 0, :],
                        start=True,
                        stop=False,
                    )
                    nc.tensor.matmul(
                        y_ps[:, f, :],
                        lhsT=vT[:, t, f, 1, :],
                        rhs=wt_bf[:, 1, :],
                        start=False,
                        stop=True,
                    )
                # y += beta' (broadcast row), in place on PSUM.
                nc.vector.tensor_add(y_ps, y_ps, beta_ap)

                o_sb = ostore.tile([P, FPT, Wd], F32, tag="o")
                # out = (u * inv_std) * (z + beta')
                nc.vector.scalar_tensor_tensor(
                    out=o_sb,
                    in0=u_in,
                    scalar=inv_std_c,
                    in1=y_ps,
                    op0=mybir.AluOpType.mult,
                    op1=mybir.AluOpType.mult,
                )
                nc.scalar.dma_start(out=out_t[t], in_=o_sb)

    return nc


def split_multiwaits(nc):
    """Walrus in this toolchain accepts at most ONE sync-wait command per
    instruction.  Tile's semaphore assignment can emit several (e.g. a DMA
    slot-reuse waits on both the previous reader's engine sem and the old
    DMA's completion lane).  Hoist all but one wait into standalone
    InstEventSemaphore instructions on the same engine stream immediately
    before the instruction — semantically identical (the sequencer performs
    the waits in order before dispatching)."""
    n_split = 0
    for f in nc.m.functions:
        for blk in f.blocks:
            new_insts = []
            for inst in blk.instructions:
                si = getattr(inst, "sync_info", None)
                if si is not None and si.on_wait and len(si.on_wait) > 1:
                    waits = list(si.on_wait)
                    for j, w in enumerate(waits[:-1]):
                        wi = mybir.InstEventSemaphore(
                            name=f"{inst.name}-hw{j}",
                            engine=inst.engine,
                            ins=[],
                            outs=[],
                        )
                        wi.sync_info = mybir.SyncInfo(on_wait=[w], on_update=[])
                        new_insts.append(wi)
                        n_split += 1
                    inst.sync_info = mybir.SyncInfo(
                        on_wait=[waits[-1]], on_update=list(si.on_update or [])
                    )
                new_insts.append(inst)
            blk.instructions[:] = new_insts
    return n_split


_NC_CACHE = None


def _get_nc():
    global _NC_CACHE
    if _NC_CACHE is None:
        nc = build_bass()
        split_multiwaits(nc)
        _NC_CACHE = nc
    return _NC_CACHE


def run(inputs, trace=False, **spmd_kwargs):
    from concourse.bass_utils import run_bass_kernel_spmd

    x = np.ascontiguousarray(np.asarray(inputs["x"], dtype=np.float32))
    W = np.ascontiguousarray(np.asarray(inputs["W"], dtype=np.float32))
    b = np.ascontiguousarray(np.asarray(inputs["b"], dtype=np.float32))
    assert x.shape == (NCORES, C2, G, Wd), x.shape

    nc = _get_nc()
    in_maps = [{"x": x[i], "W": W, "b": b} for i in range(NCORES)]
    res = run_bass_kernel_spmd(
        nc, in_maps, core_ids=list(range(NCORES)), trace=trace, **spmd_kwargs
    )
    out = np.stack([res.results[i]["out"] for i in range(NCORES)], axis=0)
    return out, res


def kernel(**inputs) -> np.ndarray:
    out, _ = run(inputs)
    return out


# revision 15
# speedup vs baseline: 1.2378x; 1.0142x over previous
"""Trainium2 Bass kernel for nn_BlockGatingUnit.

Reference computation (per batch element b of x [8, 256, 256, 256] f32):
    u, v = split(x, 2, axis=1)                  # each [128, 256, 256]
    v    = LayerNorm(v) over all non-batch dims (affine = identity)
    y    = v @ W.T + b                          # Linear along last dim
    out  = u * (y + 1)                          # [8, 128, 256, 256]

Sharding: pure data-parallel — batch dim 8 across the 8 NeuronCores, one
batch element per core, W/b replicated.  LayerNorm stats are per batch
element, so no collectives are needed.

Per-core plan (memory-bound; HBM floor = read 67MB + write 33.5MB ~ 281us
at ~358 GB/s).  LayerNorm is an affine map, so it commutes with the
Linear layer:

    out = u * (LN(v) @ W.T + b + 1)
        = (u * inv_std) * (v @ W.T + beta'),
    beta'[o] = (b[o] + 1) * std - mean * sum_w W[o, w]

which lets the matmul run on RAW (unnormalized) bf16 v and pushes all of
LayerNorm into one per-column scalar and one bf16 bias row.

  Phase 1:  casting gpsimd DMA streams v (f32 HBM -> bf16 SBUF tiles);
            bn_stats on VectorE accumulates LN stats; TensorE transposes
            each 128x128 block (w onto partitions, PE otherwise idle in
            this phase); one grouped [128,1024] PSUM->SBUF copy per tile
            on ScalarE lands the transposed bf16 v in a persistent 16.8MB
            SBUF buffer.  v is read from HBM exactly once.
  Stats:    bn_aggr + cross-partition reduce via tiny ones-matmuls ->
            inv_std column + beta' row.
  Phase 2:  pure-matmul stream on TensorE (stays at the fast p-state):
            per 128-row group, 2 bf16 matmuls against W.T chunks + a K=1
            ones-row matmul adding beta', accumulated in PSUM f32; the
            epilogue is a single fused VectorE op
            out = (u * inv_std) * y_psum; streamed u in / out via DMA.
"""

import sys

for _p in ("/opt/trn_rl_repo", "/root/.axon_site/_ro/trn_rl_repo"):
    if _p not in sys.path:
        sys.path.append(_p)

import numpy as np

import concourse.bass as bass
import concourse.tile as tile
from concourse import mybir
from concourse.masks import make_identity

F32 = mybir.dt.float32
BF16 = mybir.dt.bfloat16

EPS = 1e-5

# Per-core shard shapes (hardcoded; batch dim 8 == n_cores).
C2, G, Wd = 256, 256, 256          # x shard [C2, G, Wd]
C = C2 // 2                        # u/v channel count
ROWS = C * G                       # 32768 rows of length Wd
P = 128                            # partitions
FPT = 4                            # rows per partition per tile
TILE_ROWS = P * FPT                # 512 rows -> 1MB f32 tiles
NT = ROWS // TILE_ROWS             # 64 tiles
NCORES = 8


def build_bass():
    nc = bass.Bass()

    x_h = nc.declare_dram_parameter("x", [C2, G, Wd], F32, isOutput=False)
    w_h = nc.declare_dram_parameter("W", [Wd, Wd], F32, isOutput=False)
    b_h = nc.declare_dram_parameter("b", [Wd], F32, isOutput=False)
    o_h = nc.declare_dram_parameter("out", [C, G, Wd], F32, isOutput=True)

    x_ap = x_h[:, :, :]
    # [t, p, f, w] tiling: row = t*512 + p*4 + f, contiguous 1MB per tile.
    u_t = x_ap[0:C].rearrange("c g w -> (c g) w").rearrange(
        "(t p f) w -> t p f w", p=P, f=FPT
    )
    v_t = x_ap[C:C2].rearrange("c g w -> (c g) w").rearrange(
        "(t p f) w -> t p f w", p=P, f=FPT
    )
    out_t = o_h[:, :, :].rearrange("c g w -> (c g) w").rearrange(
        "(t p f) w -> t p f w", p=P, f=FPT
    )

    with tile.TileContext(nc) as tc:
        with (
            tc.tile_pool(name="persist", bufs=1) as persist,
            tc.tile_pool(name="consts", bufs=1) as consts,
            tc.tile_pool(name="vload", bufs=3) as vload,
            tc.tile_pool(name="vbf", bufs=3) as vbf,
            tc.tile_pool(name="uload", bufs=5) as uload,
            tc.tile_pool(name="ostore", bufs=5) as ostore,
            tc.tile_pool(name="ps", bufs=4, space="PSUM") as psall,
        ):
            # ---- constants -------------------------------------------------
            ident = consts.tile([P, P], BF16)
            make_identity(nc, ident)

            ones_col_f = consts.tile([P, 1], F32)
            nc.vector.memset(ones_col_f, 1.0)
            ones_row_f = consts.tile([1, P], F32)
            nc.vector.memset(ones_row_f, 1.0)
            ones_col_b = consts.tile([P, 1], BF16)
            nc.vector.memset(ones_col_b, 1.0)
            eps_col = consts.tile([P, 1], F32)
            nc.vector.memset(eps_col, EPS)

            # W.T in bf16: wt_bf[:, k, o] = W[o, k*128 + w_local].
            w_f32 = consts.tile([P, 2, Wd], F32)
            nc.sync.dma_start(
                out=w_f32, in_=w_h[:, :].rearrange("(m p) w -> p m w", p=P)
            )
            w_bf = consts.tile([P, 2, Wd], BF16)
            nc.scalar.copy(w_bf, w_f32)
            wt_bf = consts.tile([P, 2, Wd], BF16)
            for m in range(2):
                for k in range(2):
                    ps_w = psall.tile([P, P], F32, tag="ps")
                    # transpose as a REGULAR matmul (w_chunk.T @ I): counts
                    # as PE-busy for the clock boost, FWL-eligible LDW.
                    nc.tensor.matmul(
                        ps_w,
                        lhsT=w_bf[:, m, k * P : (k + 1) * P],
                        rhs=ident,
                        start=True,
                        stop=True,
                    )
                    nc.scalar.copy(wt_bf[:, k, m * P : (m + 1) * P], ps_w)

            # Row sums of W (= column sums of W.T): ones @ WT.
            ps_sw = psall.tile([1, Wd], F32, tag="ps")
            nc.tensor.matmul(
                ps_sw, lhsT=ones_col_b, rhs=wt_bf[:, 0, :], start=True, stop=False
            )
            nc.tensor.matmul(
                ps_sw, lhsT=ones_col_b, rhs=wt_bf[:, 1, :], start=False, stop=True
            )
            sumw_row = consts.tile([1, Wd], F32)
            nc.vector.tensor_copy(sumw_row, ps_sw)

            # b + 1 (f32 row).
            b_f32 = consts.tile([1, Wd], F32)
            nc.sync.dma_start(out=b_f32, in_=b_h[None, :])
            bp1_row = consts.tile([1, Wd], F32)
            nc.scalar.activation(
                bp1_row, b_f32, mybir.ActivationFunctionType.Identity, bias=1.0
            )

            # ---- persistent buffers ---------------------------------------
            # Transposed bf16 v: [w_local, t, f, k, r] with w on partitions.
            vT = persist.tile([P, NT, FPT, 2, P], BF16)        # 16.8 MB
            stats = persist.tile([P, NT, 2, 6], F32)           # bn_stats out

            # ---- phase 1: cast-load, stats, transpose ---------------------
            for t in range(NT):
                # Split v-loads over two DMA rings: even tiles take the
                # SWDGE (gpsimd) ring with an in-datapath f32->bf16 cast;
                # odd tiles take the HWDGE (SP) ring as f32 with an ACT
                # convert.  Each ring alone caps at ~250-260 GB/s.
                v_in = vbf.tile([P, FPT, Wd], BF16, tag="vb")
                if t % 2 == 0:
                    nc.gpsimd.dma_start(out=v_in, in_=v_t[t])
                else:
                    v_f = vload.tile([P, FPT, Wd], F32, tag="v")
                    nc.sync.dma_start(out=v_f, in_=v_t[t])
                    nc.scalar.copy(v_in, v_f)
                # bn_stats free-size limit is 512 -> two calls of 512 each.
                nc.vector.bn_stats(
                    out=stats[:, t, 0, :],
                    in_=v_in[:, 0:2, :].rearrange("p f w -> p (f w)"),
                )
                nc.vector.bn_stats(
                    out=stats[:, t, 1, :],
                    in_=v_in[:, 2:4, :].rearrange("p f w -> p (f w)"),
                )
                vt_ps = psall.tile([P, FPT, 2, P], F32, tag="ps")
                for f in range(FPT):
                    for k in range(2):
                        nc.tensor.matmul(
                            vt_ps[:, f, k, :],
                            lhsT=v_in[:, f, k * P : (k + 1) * P],
                            rhs=ident,
                            start=True,
                            stop=True,
                        )
                nc.scalar.copy(vT[:, t], vt_ps)

            # ---- stats finalize -------------------------------------------
            mvm = consts.tile([P, 3], F32)
            nc.vector.bn_aggr(
                out=mvm[:, 0:2],
                in_=stats[:, :, :, :].rearrange("p t f s -> p (t f) s"),
            )
            nc.vector.tensor_mul(mvm[:, 2:3], mvm[:, 0:1], mvm[:, 0:1])

            # Cross-partition: totals = ones_col.T @ [mean, var, mean^2],
            # then broadcast back to all partitions via ones_row.T @ totals.
            ps_tot = psall.tile([1, 3], F32, tag="ps")
            nc.tensor.matmul(
                ps_tot, lhsT=ones_col_f, rhs=mvm, start=True, stop=True
            )
            row_tot = consts.tile([1, 3], F32)
            nc.vector.tensor_copy(row_tot, ps_tot)
            ps_bc = psall.tile([P, 3], F32, tag="ps")
            nc.tensor.matmul(
                ps_bc, lhsT=ones_row_f, rhs=row_tot, start=True, stop=True
            )
            tot = consts.tile([P, 3], F32)
            nc.vector.tensor_copy(tot, ps_bc)

            mean_c = consts.tile([P, 1], F32)
            nc.vector.tensor_scalar_mul(mean_c, tot[:, 0:1], 1.0 / P)
            ex2_c = consts.tile([P, 1], F32)
            nc.vector.tensor_add(ex2_c, tot[:, 1:2], tot[:, 2:3])
            nc.vector.tensor_scalar_mul(ex2_c, ex2_c, 1.0 / P)
            msq_c = consts.tile([P, 1], F32)
            nc.vector.tensor_mul(msq_c, mean_c, mean_c)
            var_c = consts.tile([P, 1], F32)
            nc.vector.tensor_sub(var_c, ex2_c, msq_c)
            std_c = consts.tile([P, 1], F32)
            nc.scalar.activation(
                std_c, var_c, mybir.ActivationFunctionType.Sqrt, bias=eps_col
            )
            inv_std_c = consts.tile([P, 1], F32)
            nc.vector.reciprocal(inv_std_c, std_c)

            # beta'[o] = (b[o] + 1) * std - mean * sumW[o]   (bf16 row)
            beta_f = consts.tile([1, Wd], F32)
            nc.vector.tensor_scalar_mul(beta_f, bp1_row, std_c[0:1, :])
            tmp_row = consts.tile([1, Wd], F32)
            nc.vector.tensor_scalar_mul(tmp_row, sumw_row, mean_c[0:1, :])
            nc.vector.tensor_sub(beta_f, beta_f, tmp_row)
            ps_bb = psall.tile([P, Wd], F32, tag="ps")
            nc.tensor.matmul(
                ps_bb, lhsT=ones_row_f, rhs=beta_f, start=True, stop=True
            )
            beta_bc = consts.tile([P, Wd], F32)
            nc.vector.tensor_copy(beta_bc, ps_bb)
            beta_ap = bass.AP(
                tensor=beta_bc[:, :].tensor,
                offset=beta_bc[:, :].offset,
                ap=[beta_bc[:, :].ap[0], [0, FPT], [1, Wd]],
            )

            # ---- phase 2: matmul + fused epilogue -------------------------
            for t in range(NT):
                u_in = uload.tile([P, FPT, Wd], F32, tag="u")
                nc.sync.dma_start(out=u_in, in_=u_t[t])

                y_ps = psall.tile([P, FPT, Wd], F32, tag="ps")
                for f in range(FPT):
                    nc.tensor.matmul(
                        y_ps[:, f, :],
                        lhsT=vT[:, t, f, 0, :],
                        rhs=wt_bf[:, 0, :],
                        start=True,
                        stop=False,
                    )
                    nc.tensor.matmul(
                        y_ps[:, f, :],
                        lhsT=vT[:, t, f, 1, :],
                        rhs=wt_bf[:, 1, :],
                        start=False,
                        stop=True,
                    )
                # y += beta' (broadcast row), in place on PSUM.
                nc.vector.tensor_add(y_ps, y_ps, beta_ap)

                o_sb = ostore.tile([P, FPT, Wd], F32, tag="o")
                # out = (u * inv_std) * (z + beta')
                nc.vector.scalar_tensor_tensor(
                    out=o_sb,
                    in0=u_in,
                    scalar=inv_std_c,
                    in1=y_ps,
                    op0=mybir.AluOpType.mult,
                    op1=mybir.AluOpType.mult,
                )
                nc.scalar.dma_start(out=out_t[t], in_=o_sb)

    return nc


def split_multiwaits(nc):
    """Walrus in this toolchain accepts at most ONE sync-wait command per
    instruction.  Tile's semaphore assignment can emit several (e.g. a DMA
    slot-reuse waits on both the previous reader's engine sem and the old
    DMA's completion lane).  Hoist all but one wait into standalone
    InstEventSemaphore instructions on the same engine stream immediately
    before the instruction — semantically identical (the sequencer performs
    the waits in order before dispatching)."""
    n_split = 0
    for f in nc.m.functions:
        for blk in f.blocks:
            new_insts = []
            for inst in blk.instructions:
                si = getattr(inst, "sync_info", None)
                if si is not None and si.on_wait and len(si.on_wait) > 1:
                    waits = list(si.on_wait)
                    for j, w in enumerate(waits[:-1]):
                        wi = mybir.InstEventSemaphore(
                            name=f"{inst.name}-hw{j}",
                            engine=inst.engine,
                            ins=[],
                            outs=[],
                        )
                        wi.sync_info = mybir.SyncInfo(on_wait=[w], on_update=[])
                        new_insts.append(wi)
                        n_split += 1
                    inst.sync_info = mybir.SyncInfo(
                        on_wait=[waits[-1]], on_update=list(si.on_update or [])
                    )
                new_insts.append(inst)
            blk.instructions[:] = new_insts
    return n_split


_NC_CACHE = None


def _get_nc():
    global _NC_CACHE
    if _NC_CACHE is None:
        nc = build_bass()
        split_multiwaits(nc)
        _NC_CACHE = nc
    return _NC_CACHE


def run(inputs, trace=False, **spmd_kwargs):
    from concourse.bass_utils import run_bass_kernel_spmd

    x = np.ascontiguousarray(np.asarray(inputs["x"], dtype=np.float32))
    W = np.ascontiguousarray(np.asarray(inputs["W"], dtype=np.float32))
    b = np.ascontiguousarray(np.asarray(inputs["b"], dtype=np.float32))
    assert x.shape == (NCORES, C2, G, Wd), x.shape

    nc = _get_nc()
    in_maps = [{"x": x[i], "W": W, "b": b} for i in range(NCORES)]
    res = run_bass_kernel_spmd(
        nc, in_maps, core_ids=list(range(NCORES)), trace=trace, **spmd_kwargs
    )
    out = np.stack([res.results[i]["out"] for i in range(NCORES)], axis=0)
    return out, res


def kernel(**inputs) -> np.ndarray:
    out, _ = run(inputs)
    return out
